# revision 1
# baseline (speedup 1.0000x reference)
"""GAT 2-layer kernel for 8 Trainium2 NeuronCores (Bass/Tile).

Strategy (graph/data parallel per the dst-partition sharding hint):
  - Nodes live in a packed space of 50176 = 8*6272 rows (>=50000; tail pad).
    Core c owns packed rows [6272c, 6272(c+1)) = 49 dst tiles of 128.
  - Every core replicates the dense projection xw = x @ W1 (cheaper than
    all-gathering 51MB of projected features). Each core's xT input has its
    own shard's columns rotated to the front, so "local" is tiles 0..48 in
    every core's identical SPMD program. The core writes a bf16 feature
    table XW[50176, 256] (rotated order) to DRAM and keeps alpha_dst for
    its 49 local tiles in SBUF.
  - Edges (incl. self-loops) go to the core owning dst; grouped by dst
    tile, split by rotated-src half (int16 gather index limit), padded to
    128-edge chunks on a shared schedule (max chunk count over cores).
  - Per tile: dma_gather src feature rows; alpha_src dot on device;
    alpha_dst per edge via one-hot matmul; p = exp(leaky_relu(as+ad));
    segment softmax folded into one segment-sum matmul of [p*feat | p]
    against the dst one-hot; out = S/denom; xw2 = relu(out+b1) @ w2.
  - AllGather xw2 (bf16, 12.5KB/core) into a global packed table, then a
    second edge pass computes layer 2 with scalar features the same way.
"""
import numpy as np
import ml_dtypes

P = 128
N = 50000
FEAT = 256
HID = 64
HEADS = 4
NCORES = 8
TILES = 49                 # dst tiles per core
SHARD = TILES * P          # 6272 packed nodes per core
NPACK = NCORES * SHARD     # 50176
LOSPLIT = 32768            # int16 gather table split (rotated space)
NTILES_GLOBAL = NPACK // P # 392 n-tiles for the replicated projection
CMAX = 26                  # max chunks per processing round (SBUF sizing)
NEG_SLOPE = 0.2
EPS = 1e-16

bf16 = ml_dtypes.bfloat16
NB = 28                      # projection tiles per xT load batch


def set_scale(n, tiles, losplit, nb):
    """Shrink the problem for debugging. Recomputes derived globals."""
    global N, TILES, SHARD, NPACK, LOSPLIT, NTILES_GLOBAL, NB
    N, TILES, LOSPLIT, NB = n, tiles, losplit, nb
    SHARD = TILES * P
    NPACK = NCORES * SHARD
    NTILES_GLOBAL = NPACK // P


def _cdiv(a, b):
    return -(-a // b)


# ----------------------------------------------------------------------------
# host-side edge scheduling
# ----------------------------------------------------------------------------

def _build_edge_schedule(edge_index):
    """Assign edges (with self-loops) to cores by dst; per core order edges by
    (tile, half-of-rotated-src); build the shared chunk schedule.

    Returns (sched [TILES,2], per_core list of (rsrc, gsrc, dstrel)) where
    rsrc is the rotated-space src index (pads: 0 / LOSPLIT), gsrc the
    global-packed src (pads: 0), dstrel float32 (-1 for pads). All edge
    arrays are padded to the shared schedule, edge order = schedule order.
    """
    src = np.concatenate([edge_index[0], np.arange(N, dtype=np.int64)]).astype(np.int64)
    dst = np.concatenate([edge_index[1], np.arange(N, dtype=np.int64)]).astype(np.int64)
    core = dst // SHARD
    tile = (dst % SHARD) // P

    counts = np.zeros((NCORES, TILES, 2), np.int64)
    ordered = []
    for c in range(NCORES):
        m = core == c
        sc, dc, tc = src[m], dst[m], tile[m]
        rsrc = (sc - c * SHARD) % NPACK
        half = (rsrc >= LOSPLIT).astype(np.int64)
        key = tc * 2 + half
        order = np.argsort(key, kind="stable")
        ordered.append((rsrc[order], sc[order], dc[order], key[order]))
        counts[c] = np.bincount(key, minlength=TILES * 2).reshape(TILES, 2)

    sched = _cdiv(counts, P).max(axis=0)            # [TILES, 2] chunks
    total_chunks = int(sched.sum())

    per_core = []
    for c in range(NCORES):
        rsrc_s, gsrc_s, dst_s, key_s = ordered[c]
        starts = np.zeros(TILES * 2 + 1, np.int64)
        np.cumsum(counts[c].reshape(-1), out=starts[1:])
        e_rsrc = np.zeros(total_chunks * P, np.int64)
        e_gsrc = np.zeros(total_chunks * P, np.int64)
        e_dstrel = np.full(total_chunks * P, -1.0, np.float32)
        pos = 0
        for t in range(TILES):
            for h in range(2):
                k = t * 2 + h
                cnt = int(counts[c, t, h])
                n_ch = int(sched[t, h])
                sl = slice(starts[k], starts[k] + cnt)
                e_rsrc[pos : pos + cnt] = rsrc_s[sl]
                e_gsrc[pos : pos + cnt] = gsrc_s[sl]
                e_dstrel[pos : pos + cnt] = dst_s[sl] % P
                if h == 1:
                    e_rsrc[pos + cnt : pos + n_ch * P] = LOSPLIT
                pos += n_ch * P
        per_core.append((e_rsrc, e_gsrc, e_dstrel))
    return sched, per_core


def _wrap_idx(idx16):
    """[n] int16 -> [128, n/16] wrapped (i at [i%16, i//16]) + 8x replicated."""
    a = idx16.reshape(-1, 16).T
    return np.tile(a, (8, 1)).copy()


def _host_arrays(inputs):
    x = np.asarray(inputs["x"], np.float32)
    ei = np.asarray(inputs["edge_index"])
    w1 = np.asarray(inputs["w1"], np.float32)
    a_src1 = np.asarray(inputs["a_src1"], np.float32)
    a_dst1 = np.asarray(inputs["a_dst1"], np.float32)
    b1 = np.asarray(inputs["b1"], np.float32)
    w2 = np.asarray(inputs["w2"], np.float32)

    sched, per_core = _build_edge_schedule(ei)

    xT = np.zeros((FEAT, NPACK), bf16)
    xT[:, :N] = x.T.astype(bf16)

    a_dst_blk = np.zeros((FEAT, HEADS), np.float32)
    for h in range(HEADS):
        a_dst_blk[h * HID : (h + 1) * HID, h] = a_dst1[h]

    shared = dict(
        w1_bf=w1.reshape(2, P, FEAT).astype(bf16),          # rhs chunks [cc,128,256]
        w1T_bf=w1.T.reshape(2, P, FEAT).astype(bf16),       # w1T[o-chunk,128,256c]
        adblk_bf=a_dst_blk.reshape(2, P, HEADS).astype(bf16),
        a_src_row=a_src1.reshape(-1).astype(np.float32),
        b1=b1.astype(np.float32),
        w2_col=w2.reshape(2, P).astype(bf16),
        a_src2=float(np.asarray(inputs["a_src2"]).reshape(())),
        a_dst2=float(np.asarray(inputs["a_dst2"]).reshape(())),
        b2=float(np.asarray(inputs["b2"]).reshape(())),
        sched=sched,
    )

    cores = []
    for c in range(NCORES):
        e_rsrc, e_gsrc, e_dstrel = per_core[c]
        ct = e_rsrc.size // P
        rmat = e_rsrc.reshape(ct, P)
        lo_list, hi_list = [], []
        pos = 0
        for t in range(TILES):
            c_lo, c_hi = int(sched[t, 0]), int(sched[t, 1])
            lo_list.append(rmat[pos : pos + c_lo].reshape(-1))
            hi_list.append(rmat[pos + c_lo : pos + c_lo + c_hi].reshape(-1) - LOSPLIT)
            pos += c_lo + c_hi
        cores.append(dict(
            xT=np.roll(xT, -c * SHARD, axis=1).copy(),
            idx_lo=_wrap_idx(np.concatenate(lo_list).astype(np.int16)),
            idx_hi=_wrap_idx(np.concatenate(hi_list).astype(np.int16)),
            idx_l2=_wrap_idx((e_gsrc // P).astype(np.int16)),
            srcmod_pc=(e_gsrc % P).astype(np.float32).reshape(ct, P).T.copy(),
            dstrel_pc=e_dstrel.reshape(ct, P).T.copy(),
            dstrel_row=e_dstrel.reshape(1, ct * P).astype(bf16),
        ))
    return shared, cores


# ----------------------------------------------------------------------------
# numpy simulation of the exact device pipeline (layout validation)
# ----------------------------------------------------------------------------

def _simulate(shared, cores):
    sched = shared["sched"]
    w1f = np.asarray(shared["w1_bf"], np.float32).reshape(FEAT, FEAT)
    wa = w1f @ np.asarray(shared["adblk_bf"], np.float32).reshape(FEAT, HEADS)
    wa = wa.astype(bf16).astype(np.float32)
    a_src = shared["a_src_row"]
    w2f = np.asarray(shared["w2_col"], np.float32).reshape(FEAT)
    CT = int(sched.sum())

    xw2_cores = []
    l1_state = []
    for c in range(NCORES):
        m = cores[c]
        xTc = np.asarray(m["xT"], np.float32)
        xw = (xTc.T @ w1f).astype(bf16).astype(np.float32)     # rotated XW table
        ad_loc = (xTc.T[: TILES * P] @ wa)                      # local tiles only
        idx_lo = m["idx_lo"][:16].T.reshape(-1).astype(np.int64)
        idx_hi = m["idx_hi"][:16].T.reshape(-1).astype(np.int64)
        dstrel = m["dstrel_pc"].T.reshape(-1)
        S = np.zeros((TILES * P, FEAT + HEADS), np.float32)
        pos = plo = phi = 0
        for t in range(TILES):
            c_lo, c_hi = int(sched[t, 0]), int(sched[t, 1])
            nch = c_lo + c_hi
            srcs = np.concatenate([
                idx_lo[plo * P : (plo + c_lo) * P],
                idx_hi[phi * P : (phi + c_hi) * P] + LOSPLIT,
            ])
            plo += c_lo; phi += c_hi
            dr = dstrel[pos * P : (pos + nch) * P]
            pos += nch
            feat = xw[srcs]
            als = (feat.reshape(-1, HEADS, HID) * a_src.reshape(HEADS, HID)).sum(-1)
            add = ad_loc[t * P : (t + 1) * P]
            onehot = dr[:, None] == np.arange(P)[None, :]
            ad_e = onehot @ add.astype(bf16).astype(np.float32)
            e_val = als + ad_e
            e_val = np.where(e_val > 0, e_val, NEG_SLOPE * e_val)
            p = np.exp(e_val)
            msg = (feat.reshape(-1, HEADS, HID)
                   * p.astype(bf16).astype(np.float32)[:, :, None]).reshape(-1, FEAT)
            S[t * P : (t + 1) * P] += onehot.T @ np.concatenate(
                [msg.astype(bf16).astype(np.float32),
                 p.astype(bf16).astype(np.float32)], axis=1)
        denom = S[:, FEAT:] + EPS
        h1 = S[:, :FEAT] / np.repeat(denom, HID, axis=1)
        h1 = np.maximum(h1 + shared["b1"], 0.0).astype(bf16).astype(np.float32)
        xw2_cores.append(h1 @ w2f)
        l1_state.append(None)
    xw2_pack = np.concatenate(xw2_cores).astype(bf16).astype(np.float32)

    outs = []
    for c in range(NCORES):
        m = cores[c]
        dstrel = m["dstrel_pc"].T.reshape(-1)
        srcmod = m["srcmod_pc"].T.reshape(-1).astype(np.int64)
        idx_l2 = m["idx_l2"][:16].T.reshape(-1).astype(np.int64)
        xs = xw2_pack[idx_l2 * P + srcmod]
        S2 = np.zeros((TILES * P, 2), np.float32)
        pos = 0
        for t in range(TILES):
            nch = int(sched[t].sum())
            sl = slice(pos * P, (pos + nch) * P)
            pos += nch
            dr = dstrel[sl]
            xd = xw2_pack[c * SHARD + t * P : c * SHARD + (t + 1) * P]
            onehot = dr[:, None] == np.arange(P)[None, :]
            xd_e = onehot @ xd
            e2 = shared["a_src2"] * xs[sl] + shared["a_dst2"] * xd_e
            e2 = np.where(e2 > 0, e2, NEG_SLOPE * e2)
            p2 = np.exp(e2)
            m2 = np.stack([p2 * xs[sl], p2], 1).astype(bf16).astype(np.float32)
            S2[t * P : (t + 1) * P] += onehot.T @ m2
        outs.append(S2[:, 0] / (S2[:, 1] + EPS) + shared["b2"])
    return np.concatenate(outs)[:N].reshape(N, 1).astype(np.float32)


def kernel_sim(**inputs):
    shared, cores = _host_arrays(inputs)
    return _simulate(shared, cores)


# ----------------------------------------------------------------------------
# device program
# ----------------------------------------------------------------------------

def _build_program(shared, n_lo_chunks, n_hi_chunks, ct_total):
    import concourse.bacc as bacc
    import concourse.tile as tile
    import concourse.bass as bass
    import concourse.mybir as mybir
    from concourse.masks import make_identity

    sched = shared["sched"]
    dt = mybir.dt
    AF = mybir.ActivationFunctionType
    OP = mybir.AluOpType

    nc = bacc.Bacc(None, target_bir_lowering=False)

    # ---- parameters ----
    xT_d = nc.declare_dram_parameter("xT", [FEAT, NPACK], dt.bfloat16, isOutput=False)
    w1_d = nc.declare_dram_parameter("w1bf", [2, P, FEAT], dt.bfloat16, isOutput=False)
    w1T_d = nc.declare_dram_parameter("w1Tbf", [2, P, FEAT], dt.bfloat16, isOutput=False)
    adblk_d = nc.declare_dram_parameter("adblk", [2, P, HEADS], dt.bfloat16, isOutput=False)
    asrc_d = nc.declare_dram_parameter("asrc", [FEAT], dt.float32, isOutput=False)
    b1_d = nc.declare_dram_parameter("b1", [FEAT], dt.float32, isOutput=False)
    w2_d = nc.declare_dram_parameter("w2col", [2, P], dt.bfloat16, isOutput=False)
    idxlo_d = nc.declare_dram_parameter("idx_lo", [P, n_lo_chunks * 8], dt.int16, isOutput=False)
    idxhi_d = nc.declare_dram_parameter("idx_hi", [P, max(n_hi_chunks, 1) * 8], dt.int16, isOutput=False)
    idxl2_d = nc.declare_dram_parameter("idx_l2", [P, ct_total * 8], dt.int16, isOutput=False)
    srcmod_d = nc.declare_dram_parameter("srcmod_pc", [P, ct_total], dt.float32, isOutput=False)
    dstpc_d = nc.declare_dram_parameter("dstrel_pc", [P, ct_total], dt.float32, isOutput=False)
    dstrow_d = nc.declare_dram_parameter("dstrel_row", [1, ct_total * P], dt.bfloat16, isOutput=False)
    out_d = nc.declare_dram_parameter("out", [SHARD, 1], dt.float32, isOutput=True)

    XW = nc.dram_tensor("XWtab", [NPACK, FEAT], dt.bfloat16)

    a2s, a2d, b2 = shared["a_src2"], shared["a_dst2"], shared["b2"]

    assert NTILES_GLOBAL % NB == 0

    with tile.TileContext(nc) as tc:
      with (
          tc.tile_pool(name="const", bufs=1) as cpool,
          tc.tile_pool(name="dram", bufs=1, space="DRAM") as dpool,
      ):
        # ---- persistent constants / state ----
        ident = cpool.tile([P, P], dt.bfloat16)
        make_identity(nc, ident[:])
        it32 = cpool.tile([P, 1, P], dt.int32)
        nc.gpsimd.iota(it32[:, 0, :], [[1, P]], channel_multiplier=0)
        iota_f = cpool.tile([P, 1, P], dt.float32)
        nc.vector.tensor_copy(iota_f[:], it32[:])
        iota_bf = cpool.tile([P, 1, P], dt.bfloat16)
        nc.vector.tensor_copy(iota_bf[:], it32[:])
        ip32 = cpool.tile([P, 1], dt.int32)
        nc.gpsimd.iota(ip32[:], [[1, 1]], channel_multiplier=1)
        iota_p = cpool.tile([P, 1], dt.float32)
        nc.vector.tensor_copy(iota_p[:], ip32[:])
        ones1 = cpool.tile([1, P], dt.bfloat16)
        nc.gpsimd.memset(ones1[:], 1.0)
        asrc_t = cpool.tile([P, 1, FEAT], dt.float32)
        nc.sync.dma_start(asrc_t[:, 0, :], asrc_d[:].partition_broadcast(P))
        asrc_bf = cpool.tile([P, 1, FEAT], dt.bfloat16)
        nc.vector.tensor_copy(asrc_bf[:], asrc_t[:])
        b1_t = cpool.tile([P, FEAT], dt.float32)
        nc.sync.dma_start(b1_t[:], b1_d[:].partition_broadcast(P))
        w2_t = cpool.tile([P, 2], dt.bfloat16)
        nc.sync.dma_start(w2_t[:], w2_d[:].rearrange("c p -> p c"))
        ad_loc = cpool.tile([P, TILES, HEADS], dt.bfloat16)
        xw2loc = cpool.tile([P, TILES], dt.float32)
        out_sb = cpool.tile([P, TILES], dt.float32)

        xw2_bounce = dpool.tile([SHARD], dt.bfloat16)
        xw2_all = dpool.tile([NPACK], dt.bfloat16)

        # =================== phase 1: projection ===================
        with (
            tc.tile_pool(name="p1", bufs=2) as pool,
            tc.tile_pool(name="p1w", bufs=1) as wpool,
            tc.tile_pool(name="p1ps", bufs=4, space="PSUM") as psp,
            tc.tile_pool(name="p1ps2", bufs=1, space="PSUM") as psp2,
        ):
            w1a_t = wpool.tile([P, 2, FEAT + HEADS], dt.bfloat16)
            nc.sync.dma_start(w1a_t[:, :, 0:FEAT], w1_d[:].rearrange("c p f -> p c f"))
            w1T_t = wpool.tile([P, 2, FEAT], dt.bfloat16)
            nc.sync.dma_start(w1T_t[:], w1T_d[:].rearrange("c p f -> p c f"))
            adblk_t = wpool.tile([P, 2, HEADS], dt.bfloat16)
            nc.sync.dma_start(adblk_t[:], adblk_d[:].rearrange("c p h -> p c h"))

            # WA = W1 @ A_dst (on device), into w1a_t cols 256:260
            wa_ps = psp2.tile([P, 2, HEADS], dt.float32)
            for cc in range(2):
                for oc in range(2):
                    nc.tensor.matmul(
                        wa_ps[:, cc, :],
                        lhsT=w1T_t[:, oc, cc * P : (cc + 1) * P],
                        rhs=adblk_t[:, oc, :],
                        start=(oc == 0), stop=(oc == 1),
                    )
            nc.vector.tensor_copy(w1a_t[:, :, FEAT : FEAT + HEADS], wa_ps[:])

            for b in range(NTILES_GLOBAL // NB):
                xt_b = pool.tile([P, 2, NB * P], dt.bfloat16)
                for cc in range(2):
                    nc.sync.dma_start(
                        xt_b[:, cc, :], xT_d[cc * P : (cc + 1) * P, b * NB * P : (b + 1) * NB * P]
                    )
                xw_stage = pool.tile([P, NB, FEAT], dt.bfloat16)
                for i in range(NB):
                    t = b * NB + i
                    xw_ps = psp.tile([P, FEAT + HEADS], dt.float32)
                    for cc in range(2):
                        nc.tensor.matmul(
                            xw_ps[:],
                            lhsT=xt_b[:, cc, i * P : (i + 1) * P],
                            rhs=w1a_t[:, cc, :],
                            start=(cc == 0), stop=(cc == 1),
                        )
                    if i % 10 < 7:
                        nc.vector.tensor_copy(xw_stage[:, i, :], xw_ps[:, 0:FEAT])
                    else:
                        nc.scalar.copy(xw_stage[:, i, :], xw_ps[:, 0:FEAT])
                    if t < TILES:
                        nc.vector.tensor_copy(ad_loc[:, t, :], xw_ps[:, FEAT : FEAT + HEADS])
                nc.sync.dma_start(
                    XW[b * NB * P : (b + 1) * NB * P, :].rearrange("(i p) f -> p i f", p=P),
                    xw_stage[:],
                )

        # =================== phase 2: layer-1 edges ===================
        with (
            tc.tile_pool(name="eidx", bufs=1) as ipool,
            tc.tile_pool(name="e1", bufs=2) as pool,
            tc.tile_pool(name="e1s", bufs=2, space="PSUM") as psS,
            tc.tile_pool(name="e1d", bufs=2, space="PSUM") as psD,
            tc.tile_pool(name="e1a", bufs=2, space="PSUM") as psA,
            tc.tile_pool(name="e1x", bufs=1, space="PSUM") as psX,
        ):
            idxlo_t = ipool.tile([P, n_lo_chunks * 8], dt.int16)
            nc.sync.dma_start(idxlo_t[:], idxlo_d[:])
            idxhi_t = ipool.tile([P, max(n_hi_chunks, 1) * 8], dt.int16)
            nc.sync.dma_start(idxhi_t[:], idxhi_d[:])
            dstpc_t = ipool.tile([P, ct_total, 1], dt.float32)
            nc.sync.dma_start(dstpc_t[:, :, 0], dstpc_d[:])
            dstpc_bf = ipool.tile([P, ct_total, 1], dt.bfloat16)
            nc.vector.tensor_copy(dstpc_bf[:], dstpc_t[:])

            pos = plo = phi = 0
            for t in range(TILES):
                c_lo, c_hi = int(sched[t, 0]), int(sched[t, 1])
                nch = c_lo + c_hi
                S_ps = psS.tile([P, FEAT + HEADS], dt.float32)
                done = 0
                while done < nch:
                    cR = min(CMAX, nch - done)
                    r0 = pos + done                       # global chunk offset
                    # ---- gathers ----
                    G = pool.tile([P, CMAX, FEAT], dt.bfloat16, tag="G")
                    lo_a, lo_b = max(done, 0), min(done + cR, c_lo)
                    for g0 in range(lo_a, lo_b, 8):
                        g1 = min(g0 + 8, lo_b)
                        nn = g1 - g0
                        nc.gpsimd.dma_gather(
                            out_ap=G[:, g0 - done : g1 - done, :],
                            in_ap=XW[0:LOSPLIT, :],
                            idxs_ap=idxlo_t[:, (plo + g0) * 8 : (plo + g1) * 8],
                            num_idxs=nn * P, num_idxs_reg=nn * P, elem_size=FEAT,
                        )
                    hi_a, hi_b = max(done, c_lo), min(done + cR, nch)
                    for g0 in range(hi_a, hi_b, 8):
                        g1 = min(g0 + 8, hi_b)
                        nn = g1 - g0
                        nc.gpsimd.dma_gather(
                            out_ap=G[:, g0 - done : g1 - done, :],
                            in_ap=XW[LOSPLIT:NPACK, :],
                            idxs_ap=idxhi_t[:, (phi + g0 - c_lo) * 8 : (phi + g1 - c_lo) * 8],
                            num_idxs=nn * P, num_idxs_reg=nn * P, elem_size=FEAT,
                        )
                    # ---- dstrel row / one-hots ----
                    drow = pool.tile([1, CMAX * P], dt.bfloat16, tag="drow")
                    nc.sync.dma_start(drow[:1, : cR * P], dstrow_d[:1, r0 * P : (r0 + cR) * P])
                    T1 = pool.tile([P, CMAX, P], dt.bfloat16, tag="T1")
                    nc.vector.tensor_tensor(
                        out=T1[:, :cR, :], in0=iota_bf[:].to_broadcast((P, cR, P)),
                        in1=dstpc_bf[:, r0 : r0 + cR, :].to_broadcast((P, cR, P)),
                        op=OP.is_equal,
                    )
                    T2 = pool.tile([P, CMAX * P], dt.bfloat16, tag="T2")
                    for s0 in range(0, cR * P, 512):
                        s1 = min(s0 + 512, cR * P)
                        dbc = psD.tile([P, 512], dt.float32, tag="dbc")
                        nc.tensor.matmul(
                            dbc[:, : s1 - s0], lhsT=ones1[:], rhs=drow[:1, s0:s1],
                            start=True, stop=True,
                        )
                        nc.vector.tensor_scalar(
                            out=T2[:, s0:s1], in0=dbc[:, : s1 - s0],
                            scalar1=iota_p[:], scalar2=None, op0=OP.is_equal,
                        )
                    # ---- alpha_src ----
                    tmp = pool.tile([P, CMAX, FEAT], dt.bfloat16, tag="tmp")
                    nc.vector.tensor_tensor(
                        out=tmp[:, :cR, :], in0=G[:, :cR, :],
                        in1=asrc_bf[:].to_broadcast((P, cR, FEAT)), op=OP.mult,
                    )
                    als = pool.tile([P, CMAX * HEADS], dt.float32, tag="als")
                    nc.vector.tensor_reduce(
                        out=als[:, : cR * HEADS],
                        in_=tmp[:, :cR, :].rearrange("p c (h f) -> p (c h) f", h=HEADS),
                        axis=mybir.AxisListType.X, op=OP.add,
                    )
                    # ---- alpha_dst per edge ----
                    ad_ps = psA.tile([P, CMAX * HEADS], dt.float32, tag="adps")
                    for j in range(cR):
                        nc.tensor.matmul(
                            ad_ps[:, j * HEADS : (j + 1) * HEADS],
                            lhsT=T2[:, j * P : (j + 1) * P], rhs=ad_loc[:, t, :],
                            start=True, stop=True,
                        )
                    # ---- p = exp(lrelu(als+ad)) ----
                    ev = pool.tile([P, CMAX * HEADS], dt.float32, tag="ev")
                    nc.vector.tensor_tensor(
                        out=ev[:, : cR * HEADS], in0=als[:, : cR * HEADS],
                        in1=ad_ps[:, : cR * HEADS], op=OP.add,
                    )
                    lrl = pool.tile([P, CMAX * HEADS], dt.float32, tag="lrl")
                    nc.vector.scalar_tensor_tensor(
                        out=lrl[:, : cR * HEADS], in0=ev[:, : cR * HEADS],
                        scalar=NEG_SLOPE, in1=ev[:, : cR * HEADS],
                        op0=OP.mult, op1=OP.max,
                    )
                    pv = pool.tile([P, CMAX, HEADS], dt.bfloat16, tag="pv")
                    nc.scalar.activation(
                        pv[:, :cR, :].rearrange("p c h -> p (c h)"),
                        lrl[:, : cR * HEADS], AF.Exp,
                    )
                    # ---- MSGP = [p*feat | p] ----
                    MSGP = pool.tile([P, CMAX, FEAT + HEADS], dt.bfloat16, tag="MSGP")
                    nc.vector.tensor_tensor(
                        out=MSGP[:, :cR, 0:FEAT].rearrange("p c (h f) -> p c h f", h=HEADS),
                        in0=G[:, :cR, :].rearrange("p c (h f) -> p c h f", h=HEADS),
                        in1=pv[:, :cR, :].rearrange("p c (h o) -> p c h o", o=1).to_broadcast((P, cR, HEADS, HID)),
                        op=OP.mult,
                    )
                    nc.scalar.copy(MSGP[:, :cR, FEAT : FEAT + HEADS], pv[:, :cR, :])
                    # ---- segment matmul ----
                    for j in range(cR):
                        nc.tensor.matmul(
                            S_ps[:], lhsT=T1[:, j, :], rhs=MSGP[:, j, :],
                            start=(done + j == 0), stop=(done + j == nch - 1),
                        )
                    done += cR
                plo += c_lo
                phi += c_hi
                pos += nch
                # ---- tile epilogue: h1 = relu(S/denom + b1); xw2 ----
                den = pool.tile([P, HEADS], dt.float32, tag="den")
                nc.vector.tensor_scalar(
                    out=den[:], in0=S_ps[:, FEAT : FEAT + HEADS],
                    scalar1=EPS, scalar2=None, op0=OP.add,
                )
                rec = pool.tile([P, HEADS, 1], dt.float32, tag="rec")
                nc.vector.reciprocal(rec[:, :, 0], den[:])
                h1a = pool.tile([P, FEAT], dt.float32, tag="h1a")
                nc.vector.tensor_tensor(
                    out=h1a[:].rearrange("p (h f) -> p h f", h=HEADS),
                    in0=S_ps[:, 0:FEAT].rearrange("p (h f) -> p h f", h=HEADS),
                    in1=rec[:].to_broadcast((P, HEADS, HID)), op=OP.mult,
                )
                nc.vector.tensor_tensor(out=h1a[:], in0=h1a[:], in1=b1_t[:], op=OP.add)
                h1bf = pool.tile([P, FEAT], dt.bfloat16, tag="h1bf")
                nc.scalar.activation(h1bf[:], h1a[:], AF.Relu)
                hT_ps = psX.tile([P, 2, P], dt.bfloat16, tag="hT")
                for cc in range(2):
                    nc.tensor.transpose(hT_ps[:, cc, :], h1bf[:, cc * P : (cc + 1) * P], ident[:])
                hT = pool.tile([P, 2, P], dt.bfloat16, tag="hTs")
                nc.vector.tensor_copy(hT[:], hT_ps[:])
                xw2_ps = psX.tile([P, 1], dt.float32, tag="xw2ps")
                for cc in range(2):
                    nc.tensor.matmul(
                        xw2_ps[:], lhsT=hT[:, cc, :], rhs=w2_t[:, cc : cc + 1],
                        start=(cc == 0), stop=(cc == 1),
                    )
                nc.vector.tensor_copy(xw2loc[:, t : t + 1], xw2_ps[:])

            # ---- allgather xw2 ----
            xw2bf = pool.tile([P, TILES], dt.bfloat16, tag="xw2bf")
            nc.vector.tensor_copy(xw2bf[:], xw2loc[:])
            nc.sync.dma_start(xw2_bounce[:].rearrange("(t p) -> p t", p=P), xw2bf[:])
            nc.gpsimd.collective_compute(
                "AllGather", mybir.AluOpType.bypass,
                replica_groups=[list(range(NCORES))],
                ins=[xw2_bounce[:].opt()], outs=[xw2_all[:].opt()],
            )

        # =================== phase 3: layer-2 edges ===================
        XW2T = xw2_all[:].rearrange("(r k) -> r k", k=P)
        with (
            tc.tile_pool(name="eidx2", bufs=1) as ipool,
            tc.tile_pool(name="e2", bufs=2) as pool,
            tc.tile_pool(name="e2s", bufs=2, space="PSUM") as psS,
            tc.tile_pool(name="e2d", bufs=2, space="PSUM") as psD,
            tc.tile_pool(name="e2x", bufs=2, space="PSUM") as psA,
        ):
            idxl2_t = ipool.tile([P, ct_total * 8], dt.int16)
            nc.sync.dma_start(idxl2_t[:], idxl2_d[:])
            dstpc_t = ipool.tile([P, ct_total, 1], dt.float32)
            nc.sync.dma_start(dstpc_t[:, :, 0], dstpc_d[:])
            dstpc_bf = ipool.tile([P, ct_total, 1], dt.bfloat16)
            nc.vector.tensor_copy(dstpc_bf[:], dstpc_t[:])
            srcmod_t = ipool.tile([P, ct_total, 1], dt.float32)
            nc.sync.dma_start(srcmod_t[:, :, 0], srcmod_d[:])
            srcmod_bf = ipool.tile([P, ct_total, 1], dt.bfloat16)
            nc.vector.tensor_copy(srcmod_bf[:], srcmod_t[:])

            pos = 0
            for t in range(TILES):
                nch = int(sched[t].sum())
                S2_ps = psS.tile([P, 2], dt.float32)
                xd_bf = pool.tile([P, 1], dt.bfloat16, tag="xdbf")
                nc.vector.tensor_copy(xd_bf[:], xw2loc[:, t : t + 1])
                done = 0
                while done < nch:
                    cR = min(CMAX, nch - done)
                    r0 = pos + done
                    XS = pool.tile([P, CMAX, P], dt.bfloat16, tag="XS")
                    for g0 in range(0, cR, 8):
                        g1 = min(g0 + 8, cR)
                        nn = g1 - g0
                        nc.gpsimd.dma_gather(
                            out_ap=XS[:, g0:g1, :], in_ap=XW2T,
                            idxs_ap=idxl2_t[:, (r0 + g0) * 8 : (r0 + g1) * 8],
                            num_idxs=nn * P, num_idxs_reg=nn * P, elem_size=P,
                        )
                    # select col src%128: SM onehot, mult, reduce
                    SM = pool.tile([P, CMAX, P], dt.bfloat16, tag="SM")
                    nc.vector.tensor_tensor(
                        out=SM[:, :cR, :], in0=iota_bf[:].to_broadcast((P, cR, P)),
                        in1=srcmod_bf[:, r0 : r0 + cR, :].to_broadcast((P, cR, P)),
                        op=OP.is_equal,
                    )
                    nc.vector.tensor_tensor(
                        out=SM[:, :cR, :], in0=SM[:, :cR, :], in1=XS[:, :cR, :], op=OP.mult,
                    )
                    xs = pool.tile([P, CMAX], dt.float32, tag="xs")
                    nc.vector.tensor_reduce(
                        out=xs[:, :cR], in_=SM[:, :cR, :],
                        axis=mybir.AxisListType.X, op=OP.add,
                    )
                    # one-hots
                    drow = pool.tile([1, CMAX * P], dt.bfloat16, tag="drow2")
                    nc.sync.dma_start(drow[:1, : cR * P], dstrow_d[:1, r0 * P : (r0 + cR) * P])
                    T1 = pool.tile([P, CMAX, P], dt.bfloat16, tag="T12")
                    nc.vector.tensor_tensor(
                        out=T1[:, :cR, :], in0=iota_bf[:].to_broadcast((P, cR, P)),
                        in1=dstpc_bf[:, r0 : r0 + cR, :].to_broadcast((P, cR, P)),
                        op=OP.is_equal,
                    )
                    T2 = pool.tile([P, CMAX * P], dt.bfloat16, tag="T22")
                    for s0 in range(0, cR * P, 512):
                        s1 = min(s0 + 512, cR * P)
                        dbc = psD.tile([P, 512], dt.float32, tag="dbc2")
                        nc.tensor.matmul(
                            dbc[:, : s1 - s0], lhsT=ones1[:], rhs=drow[:1, s0:s1],
                            start=True, stop=True,
                        )
                        nc.vector.tensor_scalar(
                            out=T2[:, s0:s1], in0=dbc[:, : s1 - s0],
                            scalar1=iota_p[:], scalar2=None, op0=OP.is_equal,
                        )
                    # xd per edge
                    xd_ps = psA.tile([P, CMAX], dt.float32, tag="xdps")
                    for j in range(cR):
                        nc.tensor.matmul(
                            xd_ps[:, j : j + 1],
                            lhsT=T2[:, j * P : (j + 1) * P], rhs=xd_bf[:],
                            start=True, stop=True,
                        )
                    # e2 = a2s*xs + a2d*xd ; p2 = exp(lrelu(e2))
                    e2 = pool.tile([P, CMAX], dt.float32, tag="e2t")
                    nc.vector.tensor_scalar(
                        out=e2[:, :cR], in0=xd_ps[:, :cR], scalar1=a2d, scalar2=None, op0=OP.mult,
                    )
                    nc.vector.scalar_tensor_tensor(
                        out=e2[:, :cR], in0=xs[:, :cR], scalar=a2s,
                        in1=e2[:, :cR], op0=OP.mult, op1=OP.add,
                    )
                    lr2 = pool.tile([P, CMAX], dt.float32, tag="lr2")
                    nc.vector.scalar_tensor_tensor(
                        out=lr2[:, :cR], in0=e2[:, :cR],
                        scalar=NEG_SLOPE, in1=e2[:, :cR],
                        op0=OP.mult, op1=OP.max,
                    )
                    p2 = pool.tile([P, CMAX], dt.float32, tag="p2t")
                    nc.scalar.activation(p2[:, :cR], lr2[:, :cR], AF.Exp)
                    MS2 = pool.tile([P, CMAX, 2], dt.bfloat16, tag="MS2")
                    nc.vector.tensor_tensor(
                        out=MS2[:, :cR, 0], in0=p2[:, :cR], in1=xs[:, :cR], op=OP.mult,
                    )
                    nc.scalar.copy(MS2[:, :cR, 1], p2[:, :cR])
                    for j in range(cR):
                        nc.tensor.matmul(
                            S2_ps[:], lhsT=T1[:, j, :], rhs=MS2[:, j, :],
                            start=(done + j == 0), stop=(done + j == nch - 1),
                        )
                    done += cR
                pos += nch
                den2 = pool.tile([P, 1], dt.float32, tag="den2")
                nc.vector.tensor_scalar(
                    out=den2[:], in0=S2_ps[:, 1:2], scalar1=EPS, scalar2=None, op0=OP.add,
                )
                rec2 = pool.tile([P, 1], dt.float32, tag="rec2")
                nc.vector.reciprocal(rec2[:], den2[:])
                nc.vector.scalar_tensor_tensor(
                    out=out_sb[:, t : t + 1], in0=S2_ps[:, 0:1], scalar=b2,
                    in1=rec2[:], op0=OP.bypass, op1=OP.mult,
                )
            # out = S0*rec + b2  (two-step: mult then add const)
            nc.vector.tensor_scalar(
                out=out_sb[:], in0=out_sb[:], scalar1=b2, scalar2=None, op0=OP.add,
            )
            nc.sync.dma_start(out_d[:].rearrange("(t p) o -> p (t o)", p=P), out_sb[:])

    nc.finalize()
    return nc


LAST_EXEC_NS = None


def kernel(**inputs):
    import os
    from concourse.bass_utils import run_bass_kernel_spmd

    shared, cores = _host_arrays(inputs)
    sched = shared["sched"]
    n_lo = int(sched[:, 0].sum())
    n_hi = int(sched[:, 1].sum())
    ct = n_lo + n_hi

    nc = _build_program(shared, n_lo, n_hi, ct)

    in_maps = []
    for c in range(NCORES):
        m = cores[c]
        in_maps.append({
            "xT": np.asarray(m["xT"]),
            "w1bf": np.asarray(shared["w1_bf"]),
            "w1Tbf": np.asarray(shared["w1T_bf"]),
            "adblk": np.asarray(shared["adblk_bf"]),
            "asrc": shared["a_src_row"],
            "b1": shared["b1"],
            "w2col": np.asarray(shared["w2_col"]),
            "idx_lo": m["idx_lo"],
            "idx_hi": m["idx_hi"] if m["idx_hi"].size else np.zeros((P, 8), np.int16),
            "idx_l2": m["idx_l2"],
            "srcmod_pc": m["srcmod_pc"],
            "dstrel_pc": m["dstrel_pc"],
            "dstrel_row": np.asarray(m["dstrel_row"]),
            "out": None,
        })
        del in_maps[-1]["out"]

    trace = os.environ.get("GAT_TRACE", "0") == "1"
    res = run_bass_kernel_spmd(nc, in_maps, core_ids=list(range(NCORES)), trace=trace)
    global LAST_EXEC_NS
    LAST_EXEC_NS = res.exec_time_ns
    out = np.concatenate([res.results[c]["out"] for c in range(NCORES)], axis=0)
    return out[:N].astype(np.float32)


if __name__ == "__main__":
    pass

